# revision 27
# baseline (speedup 1.0000x reference)
"""AKOrN layer (attention-coupled Kuramoto oscillators) on 8 TRN2 NeuronCores.

Sharding: B*H = 2*4 = 8 (batch, head) pairs -> one pair per core.
Each core computes its head's attention matrix E = exp(scores) entirely in
SBUF (never touches HBM), runs the 5 Kuramoto steps locally, then the four
cores of each batch AllGather their cos(phases) (32KB) and every core computes
the full output projection for its batch. Host picks core 0 -> batch 0,
core 4 -> batch 1.

Self-contained: hardcodes all shapes; only imports concourse from the
container's /opt/trn_rl_repo.
"""

import math
import sys

import numpy as np

for _p in ("/opt/trn_rl_repo",):
    if _p not in sys.path:
        sys.path.insert(0, _p)

# Problem constants (from the reference nn.Module)
B, N, D, H, O = 2, 1024, 256, 4, 8
DT, STEPS = 0.1, 5
DK = D // H            # 64 head dim
P = 128                # partitions
NT = N // P            # 8 token tiles
NCORES = 8
SW = 2 * O + 1         # stationary width per j-tile: [sin | cos | ones] = 17
PI = float(np.pi)
TWO_PI = float(2 * np.pi)

_CACHE = {}


def _build_nc():
    import concourse.bacc as bacc
    import concourse.tile as tile
    import concourse.mybir as mybir
    from concourse.masks import make_identity
    from concourse.tile_rust import add_dep_helper

    f32 = mybir.dt.float32
    bf16 = mybir.dt.bfloat16
    ALU = mybir.AluOpType
    ACT = mybir.ActivationFunctionType

    nc = bacc.Bacc(
        "TRN2",
        target_bir_lowering=False,
        debug=False,
        enable_asserts=False,
        num_devices=NCORES,
    )

    # Per-core external inputs (host pre-sliced / transposed)
    xT = nc.dram_tensor("xT", [D, N], bf16, kind="ExternalInput")         # x[b].T (bf16)
    wqT = nc.dram_tensor("wqT", [D, DK], bf16, kind="ExternalInput")      # Wq_h.T
    wkT = nc.dram_tensor("wkT", [D, DK], bf16, kind="ExternalInput")      # Wk_h.T
    wpT = nc.dram_tensor("wpT", [D, O], bf16, kind="ExternalInput")       # Wp_h.T
    bpr = nc.dram_tensor("bpr", [O, 1], f32, kind="ExternalInput")        # bp_h + pi
    csdt = nc.dram_tensor("csdt", [P, 1], f32, kind="ExternalInput")      # DT*cs
    dtom = nc.dram_tensor("dtom", [P, NT * O], f32, kind="ExternalInput")  # DT*om tiled
    wob = nc.dram_tensor("wob", [H * O + 1, D], bf16, kind="ExternalInput")  # [Wo.T; bo]
    outp = nc.dram_tensor("out", [N, D], f32, kind="ExternalOutput")

    groups = [[0, 1, 2, 3], [4, 5, 6, 7]]

    with tile.TileContext(nc) as tc:
        with (
            tc.tile_pool(name="const", bufs=1) as const,
            tc.tile_pool(name="data", bufs=1) as data,
            tc.tile_pool(name="work", bufs=2) as work,
            tc.tile_pool(name="ps2", bufs=2, space="PSUM") as ps2,
            tc.tile_pool(name="ps1", bufs=1, space="PSUM") as ps1,
            tc.tile_pool(name="dram", bufs=1, space="DRAM") as dram,
        ):
            # ---------- load inputs ----------
            xtb = data.tile([P, 2 * N], bf16)       # x.T, kt-major
            for kt in range(2):
                nc.sync.dma_start(xtb[:, kt * N:(kt + 1) * N], xT[kt * P:(kt + 1) * P, :])

            wq_s = const.tile([P, 2 * DK], bf16)
            wk_s = const.tile([P, 2 * DK], bf16)
            wp_s = const.tile([P, 2 * O], bf16)
            for kt in range(2):
                nc.sync.dma_start(wq_s[:, kt * DK:(kt + 1) * DK], wqT[kt * P:(kt + 1) * P, :])
                nc.sync.dma_start(wk_s[:, kt * DK:(kt + 1) * DK], wkT[kt * P:(kt + 1) * P, :])
                nc.sync.dma_start(wp_s[:, kt * O:(kt + 1) * O], wpT[kt * P:(kt + 1) * P, :])
            bpr_s = const.tile([O, 1], f32)
            nc.sync.dma_start(bpr_s[:, :], bpr[:, :])
            csdt_s = const.tile([P, 1], f32)
            nc.sync.dma_start(csdt_s[:, :], csdt[:, :])
            dtom_s = const.tile([P, NT * O], f32)
            nc.sync.dma_start(dtom_s[:, :], dtom[:, :])
            wob_s = const.tile([H * O + 1, D], bf16)
            nc.sync.dma_start(wob_s[:, :], wob[:, :])

            ident = const.tile([P, P], f32)
            make_identity(nc, ident[:, :])
            b_mpi = const.tile([P, 1], f32)
            nc.vector.memset(b_mpi[:, :], -PI)
            b_hpi = const.tile([P, 1], f32)
            nc.vector.memset(b_hpi[:, :], PI / 2)
            # dummy Sin: makes the Sin table the first ACT_TABLE_LOAD, during
            # the DMA-wait dead time (init sins then need no reload)
            with tc.high_priority():
                sin_warm = const.tile([1, 1], f32)
                nc.vector.memset(sin_warm[:, :], 0.0)
                nc.scalar.activation(sin_warm[:, :], sin_warm[:, :], ACT.Sin)

            # ---------- collective warmup (absorbs first-call cost) ----------
            with tc.high_priority():
                agw_sb = const.tile([1, 8], f32)
                nc.vector.memset(agw_sb[:, :], 0.0)
                agw_in = dram.tile([1, 8], f32)
                agw_out = dram.tile([4, 8], f32)
                nc.gpsimd.dma_start(agw_in[:, :], agw_sb[:, :])
                nc.gpsimd.collective_compute(
                    "AllGather",
                    ALU.bypass,
                    replica_groups=groups,
                    ins=[agw_in[:, :].opt()],
                    outs=[agw_out[:, :].opt()],
                )

            # ---------- initial phases: phT [o, i] -> transpose to natural ----------
            # high priority: this chain gates the init sins -> exp table load
            with tc.high_priority():
                phtp = ps2.tile([O, N], f32, tag="big")
                for ib in range(2):
                    for kt in range(2):
                        nc.tensor.matmul(
                            phtp[:, ib * 512:(ib + 1) * 512],
                            lhsT=wp_s[:, kt * O:(kt + 1) * O],
                            rhs=xtb[:, kt * N + ib * 512: kt * N + (ib + 1) * 512],
                            start=(kt == 0),
                            stop=(kt == 1),
                        )
                pht_sb = work.tile([O, N], f32, tag="sgt")
                # + (bp + pi) while evacuating
                nc.vector.tensor_scalar(pht_sb[:, :], phtp[:, :], bpr_s[:, :],
                                        None, ALU.add)
                php = ps1.tile([P, NT * O], f32, tag="pt", bufs=2)
                for it in range(NT):
                    nc.tensor.transpose(
                        php[:, it * O:(it + 1) * O],
                        pht_sb[:, it * P:(it + 1) * P],
                        ident[0:O, 0:O],
                    )
            # shifted representation: ph' = wrap(ph + pi) into [0, 2pi).
            # HW tensor ops have no mod; use compare-and-correct (single
            # wrap is enough: |ph0| < 2pi and per-step drift < 0.15).
            ph = data.tile([P, NT * O], f32)
            wge = work.tile([P, NT * O], f32, tag="wge")

            nc.vector.tensor_scalar(wge[:, :], php[:, :], TWO_PI, None, ALU.is_ge)
            nc.vector.scalar_tensor_tensor(
                wge[:, :], php[:, :], 0.0, wge[:, :], ALU.is_lt, ALU.subtract)
            nc.vector.scalar_tensor_tensor(
                ph[:, :], wge[:, :], TWO_PI, php[:, :], ALU.mult, ALU.add)

            # ---------- q/k projections (bf16) ----------
            qt = data.tile([DK, N], bf16)
            ktt = data.tile([DK, N], bf16)
            for dst, w_s in ((qt, wq_s), (ktt, wk_s)):
                for ib in range(2):
                    pq = ps2.tile([DK, 512], f32, tag="pc")
                    for kt in range(2):
                        nc.tensor.matmul(
                            pq[:, :],
                            lhsT=w_s[:, kt * DK:(kt + 1) * DK],
                            rhs=xtb[:, kt * N + ib * 512: kt * N + (ib + 1) * 512],
                            start=(kt == 0),
                            stop=(kt == 1),
                        )
                    nc.scalar.copy(dst[:, ib * 512:(ib + 1) * 512], pq[:, :])

            # ---------- scores + exp -> E^T (bf16, [j_p, jt-major i]) ----------
            etb = data.tile([P, NT * N], bf16)
            exp_insts = []
            for jt in range(NT):
                psc = ps2.tile([P, N], f32, tag="big")
                for ib in range(2):
                    nc.tensor.matmul(
                        psc[:, ib * 512:(ib + 1) * 512],
                        lhsT=ktt[:, jt * P:(jt + 1) * P],
                        rhs=qt[:, ib * 512:(ib + 1) * 512],
                        start=True,
                        stop=True,
                    )
                e_i = nc.scalar.activation(etb[:, jt * N:(jt + 1) * N], psc[:, :],
                                           ACT.Exp, scale=1.0 / math.sqrt(DK))
                exp_insts.append(e_i)

            # ---------- stationary sin/cos/ones tiles ----------
            # double-buffered by step parity AND split lo/hi so next step's
            # first accumulations only depend on the lo-half sins
            HBT = NT // 2
            scw_al = data.tile([P, HBT * SW], bf16)
            scw_ah = data.tile([P, HBT * SW], bf16)
            scw_bl = data.tile([P, HBT * SW], bf16)
            scw_bh = data.tile([P, HBT * SW], bf16)
            scws = [(scw_al, scw_ah), (scw_bl, scw_bh)]
            scw3s = [tuple(t[:, :].rearrange("p (t w) -> p t w", w=SW) for t in pair)
                     for pair in scws]
            for pair in scws:
                for t in pair:
                    for jt in range(HBT):
                        nc.vector.memset(t[:, jt * SW + 2 * O: (jt + 1) * SW], 1.0)

            ph3 = ph[:, :].rearrange("p (t o) -> p t o", o=O)
            tmp = work.tile([P, NT * O], f32, tag="tmp")
            tmp3 = tmp[:, :].rearrange("p (t o) -> p t o", o=O)

            # s = sin(ph'-pi); c = cos(ph'-pi) = sin(pi/2 - |ph'-pi|)
            nc.scalar.activation(tmp3, ph3, ACT.Abs, bias=b_mpi[:, :], scale=1.0)
            ci = None
            for hb in range(2):
                hs = slice(hb * HBT, (hb + 1) * HBT)
                s3h = scw3s[0][hb][:, :, 0:O]
                c3h = scw3s[0][hb][:, :, O:2 * O]
                nc.scalar.activation(s3h, ph3[:, hs, :], ACT.Sin, bias=b_mpi[:, :], scale=1.0)
                ci = nc.scalar.activation(c3h, tmp3[:, hs, :], ACT.Sin,
                                          bias=b_hpi[:, :], scale=-1.0)
            # ACT stream grouped by table set: [init sins] -> [exps] -> [step sins]
            for k, e_i in enumerate(exp_insts):
                add_dep_helper(e_i.ins, ci.ins, sync=(k == 0),
                               reason="group ACT ops by table set")

            # ---------- Kuramoto steps ----------
            gfull = data.tile([P, NT * O], f32)
            gfull3 = gfull[:, :].rearrange("p (t o) -> p t o", o=O)
            rinv = data.tile([P, NT], f32)
            dtom3 = dtom_s[:, :].rearrange("p (t o) -> p t o", o=O)
            HB = NT // 2  # it-tiles per half
            cnat_l = data.tile([P, HB * O], bf16)
            cnat_h = data.tile([P, HB * O], bf16)
            cnats = [cnat_l, cnat_h]

            phd = data.tile([P, NT * O], f32)
            phd3 = phd[:, :].rearrange("p (t o) -> p t o", o=O)

            def half_update(step, hb, pt3, scw3, scw3_nxt):
                # ph holds the PRE-wrap phase after this update; the sins run
                # on it directly (HW Sin is accurate ~0.15 rad past +-pi) and
                # the wrap into [0, 2pi) is deferred off the critical path.
                hs = slice(hb * HB, (hb + 1) * HB)
                es_v = pt3[:, hs, 0:O]
                ec_v = pt3[:, hs, O:2 * O]
                sv = scw3[hb][:, :, 0:O]
                cv = scw3[hb][:, :, O:2 * O]
                ph_h = ph3[:, hs, :]
                if step == 0:
                    nc.vector.reciprocal(rinv[:, hs, None], pt3[:, hs, 2 * O:SW])
                    nc.vector.tensor_scalar(
                        gfull3[:, hs, :],
                        rinv[:, hs, None].to_broadcast((P, HB, O)),
                        csdt_s[:, :], None, ALU.mult,
                    )
                t1 = work.tile([P, HB * O], f32, tag="t1", name="t1")
                t13 = t1[:, :].rearrange("p (t o) -> p t o", o=O)
                t2 = work.tile([P, HB * O], f32, tag="t2", name="t2")
                t23 = t2[:, :].rearrange("p (t o) -> p t o", o=O)
                nc.vector.tensor_tensor(t13, cv, es_v, ALU.mult)
                nc.vector.tensor_tensor(t23, sv, ec_v, ALU.mult)
                nc.vector.tensor_tensor(t13, t13, t23, ALU.subtract)
                nc.vector.tensor_tensor(t13, t13, gfull3[:, hs, :], ALU.mult)
                nc.vector.tensor_tensor(ph_h, t13, phd3[:, hs, :], ALU.add)
                if step < STEPS - 1:
                    sv_n = scw3_nxt[hb][:, :, 0:O]
                    cv_n = scw3_nxt[hb][:, :, O:2 * O]
                    nc.scalar.activation(sv_n, ph_h, ACT.Sin, bias=b_mpi[:, :], scale=1.0)
                    nc.scalar.activation(tmp3[:, hs, :], ph_h, ACT.Abs,
                                         bias=b_mpi[:, :], scale=1.0)
                    nc.scalar.activation(cv_n, tmp3[:, hs, :], ACT.Sin,
                                         bias=b_hpi[:, :], scale=-1.0)
                else:
                    # final sig = cos(phases), per half (starts AG sooner)
                    cn3 = cnats[hb][:, :].rearrange("p (t o) -> p t o", o=O)
                    nc.scalar.activation(tmp3[:, hs, :], ph_h, ACT.Abs,
                                         bias=b_mpi[:, :], scale=1.0)
                    nc.scalar.activation(cn3, tmp3[:, hs, :], ACT.Sin,
                                         bias=b_hpi[:, :], scale=-1.0)

            def half_wrap(hb):
                # wrap ph into [0, 2pi): ph += 2pi*([ph<0] - [ph>=2pi]).
                # Runs after the sins have consumed the pre-wrap value.
                hs = slice(hb * HB, (hb + 1) * HB)
                ph_h = ph3[:, hs, :]
                tw = work.tile([P, HB * O], f32, tag="tw", name="tw")
                tw3 = tw[:, :].rearrange("p (t o) -> p t o", o=O)
                nc.vector.tensor_scalar(tw3, ph_h, TWO_PI, None, ALU.is_ge)
                nc.vector.scalar_tensor_tensor(
                    tw3, ph_h, 0.0, tw3, ALU.is_lt, ALU.subtract)
                nc.vector.scalar_tensor_tensor(
                    ph_h, tw3, TWO_PI, ph_h, ALU.mult, ALU.add)

            for step in range(STEPS):
                scw_pair = scws[step % 2]
                scw3 = scw3s[step % 2]
                scw3_nxt = scw3s[(step + 1) % 2]
                # off-critical-path: phd = ph + DT*omega (DVE, during MM stream)
                nc.vector.tensor_tensor(phd[:, :], ph[:, :], dtom_s[:, :], ALU.add)

                def scw_sl(jt):
                    t = scw_pair[jt // HBT]
                    j = jt % HBT
                    return t[:, j * SW:(j + 1) * SW]

                pt = ps1.tile([P, NT * SW], f32, tag="pt", bufs=2)
                pt3 = pt[:, 0:NT * SW].rearrange("p (t w) -> p t w", w=SW)

                # ib0 matmuls
                pc0 = ps2.tile([SW, 512], f32, tag="pc", name="pc0")
                for jt in range(NT):
                    nc.tensor.matmul(
                        pc0[:, :],
                        lhsT=scw_sl(jt),
                        rhs=etb[:, jt * N: jt * N + 512],
                        start=(jt == 0),
                        stop=(jt == NT - 1),
                    )
                ce0 = work.tile([SW, 512], f32, tag="ce0", name="ce0")
                nc.vector.tensor_copy(ce0[:, :], pc0[:, :])
                # ib1 matmuls with ib0's transposes + update woven in: the
                # PE reaches the transposes only after ce0 is long ready, and
                # the DVE/ACT half-0 update overlaps the rest of ib1's MMs
                pc1 = ps2.tile([SW, 512], f32, tag="pc", name="pc1")
                for jt in range(NT):
                    nc.tensor.matmul(
                        pc1[:, :],
                        lhsT=scw_sl(jt),
                        rhs=etb[:, jt * N + 512: jt * N + 1024],
                        start=(jt == 0),
                        stop=(jt == NT - 1),
                    )
                    if jt == 2:
                        for itl in range(HB):
                            nc.tensor.transpose(
                                pt[:, itl * SW:(itl + 1) * SW],
                                ce0[:, itl * P:(itl + 1) * P],
                                ident[0:SW, 0:SW],
                            )
                    if jt == 3:
                        half_update(step, 0, pt3, scw3, scw3_nxt)
                ce1 = work.tile([SW, 512], f32, tag="ce1", name="ce1")
                nc.vector.tensor_copy(ce1[:, :], pc1[:, :])
                for itl in range(HB):
                    it = HB + itl
                    nc.tensor.transpose(
                        pt[:, it * SW:(it + 1) * SW],
                        ce1[:, itl * P:(itl + 1) * P],
                        ident[0:SW, 0:SW],
                    )
                half_update(step, 1, pt3, scw3, scw3_nxt)
                if step < STEPS - 1:
                    half_wrap(0)
                    half_wrap(1)

            # ---------- sig^T -> AllGather ----------
            identb = const.tile([P, P], bf16)
            nc.vector.tensor_copy(identb[:, :], ident[:, :])
            ag_in = dram.tile([O, N], bf16)
            ag_out = dram.tile([H * O, N], bf16)
            for hb in range(2):
                psth = ps2.tile([O, 512], bf16, tag="pc", name="psth")
                for itl in range(HB):
                    it = hb * HB + itl
                    nc.tensor.transpose(
                        psth[:, itl * P:(itl + 1) * P],
                        cnats[hb][:, itl * O:(itl + 1) * O],
                        identb[:, :],
                    )
                sgth = work.tile([O, 512], bf16, tag="sgt2", name="sgth")
                nc.vector.tensor_copy(sgth[:, :], psth[:, :])
                nc.sync.dma_start(ag_in[:, hb * 512:(hb + 1) * 512], sgth[:, :])
            nc.gpsimd.collective_compute(
                "AllGather",
                ALU.bypass,
                replica_groups=groups,
                ins=[ag_in[:, :].opt()],
                outs=[ag_out[:, :].opt()],
            )
            sgf = data.tile([H * O + 1, N], bf16)
            nc.sync.dma_start(sgf[0:H * O, :], ag_out[:, :])
            nc.vector.memset(sgf[H * O:H * O + 1, :], 1.0)

            # ---------- output projection (two it-tiles per PSUM bank) ----------
            for ip in range(NT // 2):
                po = ps2.tile([P, 2 * D], f32, tag="pc")
                for k in range(2):
                    it = ip * 2 + k
                    nc.tensor.matmul(po[:, k * D:(k + 1) * D],
                                     lhsT=sgf[:, it * P:(it + 1) * P],
                                     rhs=wob_s[:, :], start=True, stop=True)
                ot = work.tile([P, 2 * D], f32, tag="ot")
                if ip % 2 == 0:
                    nc.vector.tensor_copy(ot[:, :], po[:, :])
                else:
                    nc.scalar.copy(ot[:, :], po[:, :])
                nc.sync.dma_start(
                    outp[ip * 2 * P:(ip + 1) * 2 * P, :].rearrange(
                        "(k p) d -> p k d", k=2),
                    ot[:, :].rearrange("p (k d) -> p k d", k=2))

    nc.compile()
    return nc


def get_nc():
    if "nc" not in _CACHE:
        _CACHE["nc"] = _build_nc()
    return _CACHE["nc"]


def make_in_maps(x, Wq, Wk, Wp, bp, Wo, bo, omega, coupling_scale):
    import concourse.mybir as mybir

    bf16 = mybir.dt.np(mybir.dt.bfloat16)
    x = np.asarray(x, np.float32)
    Wq = np.asarray(Wq, np.float32)
    Wk = np.asarray(Wk, np.float32)
    Wp = np.asarray(Wp, np.float32)
    bp = np.asarray(bp, np.float32)
    Wo = np.asarray(Wo, np.float32)
    bo = np.asarray(bo, np.float32)
    omega = np.asarray(omega, np.float32)
    cs = float(np.asarray(coupling_scale, np.float32))

    wob_full = np.ascontiguousarray(
        np.concatenate([Wo.T, bo[None, :]], axis=0)).astype(bf16)
    csdt_full = np.full((P, 1), DT * cs, np.float32)

    in_maps = []
    for c in range(NCORES):
        b, h = c // H, c % H
        in_maps.append({
            "xT": np.ascontiguousarray(x[b].T).astype(bf16),
            "wqT": np.ascontiguousarray(Wq[h * DK:(h + 1) * DK, :].T).astype(bf16),
            "wkT": np.ascontiguousarray(Wk[h * DK:(h + 1) * DK, :].T).astype(bf16),
            "wpT": np.ascontiguousarray(Wp[h * O:(h + 1) * O, :].T).astype(bf16),
            "bpr": np.ascontiguousarray(
                (bp[h * O:(h + 1) * O] + np.pi)[:, None], np.float32),
            "csdt": csdt_full,
            "dtom": np.ascontiguousarray(
                np.tile((DT * omega[h])[None, :], (P, NT)), np.float32),
            "wob": wob_full,
        })
    return in_maps


def run_on_hw(in_maps, trace=False):
    from concourse.bass_utils import run_bass_kernel_spmd

    nc = get_nc()
    return run_bass_kernel_spmd(nc, in_maps, core_ids=list(range(NCORES)), trace=trace)


def kernel(x, Wq, Wk, Wp, bp, Wo, bo, omega, coupling_scale):
    in_maps = make_in_maps(x, Wq, Wk, Wp, bp, Wo, bo, omega, coupling_scale)
    res = run_on_hw(in_maps, trace=False)
    out = np.stack([res.results[0]["out"], res.results[H]["out"]], axis=0)
    return np.ascontiguousarray(out, np.float32)
